# revision 10
# baseline (speedup 1.0000x reference)
"""Trainium2 Bass kernel for nn_FACoef.

Computes, for each batch b of x (B, 512, 512):
    out[b] = sum_{i<3, j<3} coef[i,j] * sum_elems((x_b^(i+2)) ** (j+1)) / (N*N)^(i+j+2)

Strategy (pure data parallel, 8 batches per core on 8 NeuronCores):
  Work with y = x^T (host passes x^T as a second DMA input - pure input
  layout prep).  y^k = (x^k)^T and the elementwise power-sums are
  transpose invariant, so the chain y2 = y@y, y3 = y@y2, y4 = y@y3 runs
  on the PE with natural-layout x as the stationary operand (lhsT = x)
  and the previous result as the moving operand - no on-device
  transposes at all.

  Matmuls run in float32r (single-pass FP22 multiply, ~1 col/cycle);
  the PE pace of 16 x 512-col matmuls per chain step (~3.6us) is the
  roofline.  Batches are processed in software-pipelined PAIRS,
  alternating the two batches' chain steps so each step's PSUM->SBUF
  copy hides under the other batch's matmuls.

  Elementwise work per result matrix y (128x2048 row-block layout) is
  split across three engines so none exceeds the PE pace:
    - ScalarE: psum->sbuf Copy (+fused s1 accum), Square on cols [0:S)
    - VectorE: Square via scalar_tensor_tensor on [S:S+V) then ONE
      merged affine_mul_reduce over all 2048 cols -> s3
    - GpSimd:  Square via tensor_tensor on [S+V:2048) + total
      tensor_reduce(XYZWC) of that slice -> s2 part (gpsimd has no
      fused accumulator and cannot touch PSUM)
  All square slices write one shared t2 tile so the cube reduce is a
  single DVE op.

  Head: the first two batches' input chunks are issued as sub-chunk
  DMAs round-robined over two issuing queues so several of the 16 HW
  DMA engines pull on each chunk concurrently (a whole 256KB chunk on
  one engine takes ~11.6us; split, it lands in a fraction).

  Tail: the very last batch's x^4 step skips the psum->sbuf copy; its
  squares run on ScalarE straight from PSUM and the cube reduces use
  AMR with in1=PSUM.  s1(x^4) = r1 . u3, where r1 = row sums of x
  (host-precomputed input, like xt) and u3 = row sums of x^3 obtained
  by splitting the x^3 copy into per-row-block activations whose fused
  accumulators ARE u3.  This cuts the post-last-matmul critical path
  from ~6.6us to ~3.5us.

  Host reduces partitions and applies coef/norm in float64.
"""

import numpy as np

import concourse.bacc as bacc
import concourse.mybir as mybir
import concourse.tile as tile
from concourse.bass_utils import run_bass_kernel_spmd

N = 512
RB = 4  # row blocks of 128
BPC = 8  # batches per core
NCORES = 8
ROWS = 3

# steady-state square-slice split (cols of 2048): scalar / vector / gpsimd
SQ_STEADY = (512, 768, 768)
# the final batch's x^3 step: scalar does the 4-way split copy instead
SQ_SPLITCOPY = (0, 1024, 1024)

# head DMA sub-chunk split per batch index (others use whole-chunk DMAs)
HEAD_SPLIT = {0: 2, 1: 2}

N_WARMUP = 20  # bf16 warmup matmuls of 256 cols each

FP32 = mybir.dt.float32
FP32R = mybir.dt.float32r
BF16 = mybir.dt.bfloat16
AF = mybir.ActivationFunctionType
ALU = mybir.AluOpType


def build_nc():
    nc = bacc.Bacc(None, target_bir_lowering=False)
    x_ext = nc.declare_dram_parameter("x", [BPC, N, N], FP32, isOutput=False)
    xt_ext = nc.declare_dram_parameter("xt", [BPC, N, N], FP32, isOutput=False)
    r1_ext = nc.declare_dram_parameter("r1", [128, RB], FP32, isOutput=False)
    # acc_s: per (batch,step) ci: [s1, s2a]   (scalar-written)
    # acc_v: per ci: [s2b, s3]                (vector-written)
    # acc_gs: [1, ci]: s2 part from gpsimd's total reduce
    # acc_u (final batch): [0:4]=u3, [4]=dot(r1,u3), [5:8]=tail s2 parts,
    # [8:11]=tail s3 parts
    acc_s_ext = nc.declare_dram_parameter("acc_s", [128, BPC * ROWS * 2], FP32, isOutput=True)
    acc_v_ext = nc.declare_dram_parameter("acc_v", [128, BPC * ROWS * 2], FP32, isOutput=True)
    acc_gs_ext = nc.declare_dram_parameter("acc_gs", [1, BPC * ROWS], FP32, isOutput=True)
    acc_u_ext = nc.declare_dram_parameter("acc_u", [128, 12], FP32, isOutput=True)

    with tile.TileContext(nc) as tc:
        with (
            tc.tile_pool(name="xpool", bufs=16) as xpool,
            tc.tile_pool(name="ycpool", bufs=16) as ycpool,
            tc.tile_pool(name="ypool", bufs=8) as ypool,
            tc.tile_pool(name="tpool", bufs=3) as tpool,
            tc.tile_pool(name="accpool", bufs=1) as accpool,
            tc.tile_pool(name="ps", bufs=2, space="PSUM") as pspool,
        ):
            acc_s = accpool.tile([128, BPC * ROWS * 2], FP32)
            acc_v = accpool.tile([128, BPC * ROWS * 2], FP32)
            acc_gs = accpool.tile([1, BPC * ROWS], FP32)
            acc_u = accpool.tile([128, 12], FP32)
            r1 = accpool.tile([128, RB], FP32)
            nc.sync.dma_start(out=r1, in_=r1_ext[:, :])

            # HAM warmup: short bf16 matmuls keep the PE busy while the
            # first input chunks DMA in, lifting the clock gate to 2.4 GHz.
            w_lhs = accpool.tile([128, 128], BF16)
            w_rhs = accpool.tile([128, 256], BF16)
            nc.gpsimd.memset(w_lhs, 1.0)
            nc.gpsimd.memset(w_rhs, 1.0)
            # the final batch's special paths leave a few acc slots unwritten
            nc.gpsimd.memset(acc_s[:, 2 * (BPC - 1) * ROWS :], 0.0)
            nc.gpsimd.memset(acc_v[:, 2 * (BPC - 1) * ROWS :], 0.0)
            nc.gpsimd.memset(acc_gs, 0.0)
            nc.gpsimd.memset(acc_u, 0.0)
            ps_warm = pspool.tile([128, RB * N], FP32, tag="ps")
            for _ in range(N_WARMUP):
                nc.tensor.matmul(
                    ps_warm[:, 0:256], lhsT=w_lhs, rhs=w_rhs, start=True, stop=True
                )

            dma_engs = [nc.sync, nc.scalar]
            dma_rr = [0]

            def load_batch(b):
                """Issue input DMAs for batch b; returns (x chunks, xt chunks).

                Chunks of head batches are split into sub-chunk DMAs issued
                round-robin over two queues so several HW DMA engines pull
                on one chunk concurrently.
                """
                nsplit = HEAD_SPLIT.get(b, 1)
                csz = N // nsplit
                sbx_c, yc_c = [], []
                for kk in range(RB):
                    sc = xpool.tile([128, N], FP32R, tag="sbx")
                    yc = ycpool.tile([128, N], FP32R, tag="yc")
                    rows = slice(128 * kk, 128 * (kk + 1))
                    for p in range(nsplit):
                        cols = slice(csz * p, csz * (p + 1))
                        eng = dma_engs[dma_rr[0] % len(dma_engs)]
                        dma_rr[0] += 1
                        eng.dma_start(
                            out=sc[:, cols],
                            in_=x_ext[b, rows, cols].bitcast(FP32R),
                        )
                        eng = dma_engs[dma_rr[0] % len(dma_engs)]
                        dma_rr[0] += 1
                        eng.dma_start(
                            out=yc[:, cols],
                            in_=xt_ext[b, rows, cols].bitcast(FP32R),
                        )
                    sbx_c.append(sc)
                    yc_c.append(yc)
                return sbx_c, yc_c

            def chain_step(sbx_c, ycur, ci, first, shares=SQ_STEADY,
                           split_copy=False, tail=False):
                """One matmul group + elementwise power-sums; returns new y."""
                psY = pspool.tile([128, RB * N], FP32, tag="ps")
                if first:
                    for kk in range(RB):
                        for m in range(RB):
                            nc.tensor.matmul(
                                psY[:, m * N : (m + 1) * N],
                                lhsT=sbx_c[kk][:, 128 * m : 128 * (m + 1)],
                                rhs=ycur[kk][:, :],
                                start=(kk == 0),
                                stop=(kk == RB - 1),
                            )
                else:
                    for m in range(RB):
                        for kk in range(RB):
                            nc.tensor.matmul(
                                psY[:, m * N : (m + 1) * N],
                                lhsT=sbx_c[kk][:, 128 * m : 128 * (m + 1)],
                                rhs=ycur[:, kk * N : (kk + 1) * N],
                                start=(kk == 0),
                                stop=(kk == RB - 1),
                            )

                t2 = tpool.tile([128, RB * N], FP32, tag="t2")
                if tail:
                    # final batch x^4: no sbuf copy.  Scalar squares chunks
                    # straight from PSUM; DVE cubes trail each chunk with
                    # in1=PSUM (one PSUM input per instruction is legal).
                    # s1 comes from r1 . u3 (emitted by the caller).
                    y_f = psY[:, :].bitcast(FP32)
                    bounds = [0, 768, 1536, 2048]
                    for j in range(3):
                        lo, hi = bounds[j], bounds[j + 1]
                        nc.scalar.activation(
                            t2[:, lo:hi],
                            y_f[:, lo:hi],
                            AF.Square,
                            accum_out=acc_u[:, 5 + j : 6 + j],
                        )
                        dj = tpool.tile([128, 1], FP32, tag=f"d{j}")
                        nc.vector.affine_mul_reduce(
                            out=dj.broadcast_to((128, hi - lo)),
                            accum_out=acc_u[:, 8 + j : 9 + j],
                            in0=t2[:, lo:hi],
                            in1=y_f[:, lo:hi],
                            scale=1.0,
                            bias=0.0,
                        )
                    return None

                ysb = ypool.tile([128, RB * N], FP32R, tag="y")
                if split_copy:
                    # x^3 of the final batch: per-m-block copies whose fused
                    # accumulators are the row sums u3 (s1 = host-sum of them)
                    for m in range(RB):
                        nc.scalar.activation(
                            ysb[:, m * N : (m + 1) * N],
                            psY[:, m * N : (m + 1) * N],
                            AF.Copy,
                            accum_out=acc_u[:, m : m + 1],
                        )
                else:
                    nc.scalar.activation(
                        ysb, psY, AF.Copy, accum_out=acc_s[:, 2 * ci : 2 * ci + 1]
                    )
                y_f = ysb[:, :].bitcast(FP32)

                # squares into one shared t2 tile, up to three engines
                S, V, G = shares
                if S:
                    nc.scalar.activation(
                        t2[:, 0:S],
                        y_f[:, 0:S],
                        AF.Square,
                        accum_out=acc_s[:, 2 * ci + 1 : 2 * ci + 2],
                    )
                nc.vector.scalar_tensor_tensor(
                    out=t2[:, S : S + V],
                    in0=y_f[:, S : S + V],
                    scalar=1.0,
                    in1=y_f[:, S : S + V],
                    op0=ALU.mult,
                    op1=ALU.mult,
                    accum_out=acc_v[:, 2 * ci : 2 * ci + 1],
                )
                if G:
                    nc.gpsimd.tensor_tensor(
                        out=t2[:, S + V :],
                        in0=y_f[:, S + V :],
                        in1=y_f[:, S + V :],
                        op=ALU.mult,
                    )
                    nc.gpsimd.tensor_reduce(
                        out=acc_gs[:, ci : ci + 1],
                        in_=t2[:, S + V :],
                        axis=mybir.AxisListType.XYZWC,
                        op=ALU.add,
                    )
                # single merged cube reduce: s3 = sum(t2 * y)
                t3d = tpool.tile([128, 1], FP32, tag="t3d")
                nc.vector.affine_mul_reduce(
                    out=t3d.broadcast_to((128, RB * N)),
                    accum_out=acc_v[:, 2 * ci + 1 : 2 * ci + 2],
                    in0=t2,
                    in1=y_f,
                    scale=1.0,
                    bias=0.0,
                )
                return ysb

            npairs = BPC // 2
            last_b = BPC - 1
            loaded = {0: (load_batch(0), load_batch(1))}
            for pair in range(npairs):
                ba, bb = 2 * pair, 2 * pair + 1
                (sbx_a, ycur_a), (sbx_b, ycur_b) = loaded.pop(pair)
                if pair + 1 < npairs:
                    loaded[pair + 1] = (
                        load_batch(2 * pair + 2),
                        load_batch(2 * pair + 3),
                    )
                for k in range(ROWS):
                    ycur_a = chain_step(sbx_a, ycur_a, ba * ROWS + k, k == 0)
                    is_last = bb == last_b
                    ycur_b = chain_step(
                        sbx_b,
                        ycur_b,
                        bb * ROWS + k,
                        k == 0,
                        shares=SQ_SPLITCOPY if (is_last and k == 1) else SQ_STEADY,
                        split_copy=(is_last and k == 1),
                        tail=(is_last and k == 2),
                    )
                if bb == last_b:
                    # s1(x^4) = r1 . u3 (both [128, RB]; host sums partitions)
                    du = tpool.tile([128, RB], FP32, tag="du")
                    nc.vector.scalar_tensor_tensor(
                        out=du,
                        in0=acc_u[:, 0:RB],
                        scalar=1.0,
                        in1=r1,
                        op0=ALU.mult,
                        op1=ALU.mult,
                        accum_out=acc_u[:, 4:5],
                    )
                cs0, cs1 = 2 * ba * ROWS, 2 * (bb + 1) * ROWS
                nc.sync.dma_start(out=acc_s_ext[:, cs0:cs1], in_=acc_s[:, cs0:cs1])
                nc.sync.dma_start(out=acc_v_ext[:, cs0:cs1], in_=acc_v[:, cs0:cs1])
                if bb == last_b:
                    nc.sync.dma_start(out=acc_gs_ext[:, :], in_=acc_gs[:, :])
                    nc.sync.dma_start(out=acc_u_ext[:, :], in_=acc_u[:, :])

    nc.finalize()
    return nc


_NC_CACHE = None


def get_nc():
    global _NC_CACHE
    if _NC_CACHE is None:
        _NC_CACHE = build_nc()
    return _NC_CACHE


def combine_partials(acc_s, acc_v, acc_gs, acc_u, coef, out, base):
    """Reduce per-partition partials and apply coef/norm in float64."""
    s = acc_s.astype(np.float64).sum(axis=0)
    v = acc_v.astype(np.float64).sum(axis=0)
    g = acc_gs.astype(np.float64).reshape(-1)
    u = acc_u.astype(np.float64).sum(axis=0)
    norm_pow = (
        np.arange(3)[None, :] + np.arange(ROWS)[:, None] + 2
    ).astype(np.float64)
    w = coef.astype(np.float64) / (float(N * N) ** norm_pow)  # (ROWS, 3)
    last_b = BPC - 1
    for b in range(BPC):
        acc = 0.0
        for i in range(ROWS):
            ci = b * ROWS + i
            if b == last_b and i == 1:
                s1 = u[0] + u[1] + u[2] + u[3]
                s2 = v[2 * ci] + g[ci]
                s3 = v[2 * ci + 1]
            elif b == last_b and i == 2:
                s1 = u[4]
                s2 = u[5] + u[6] + u[7]
                s3 = u[8] + u[9] + u[10]
            else:
                s1 = s[2 * ci]
                s2 = s[2 * ci + 1] + v[2 * ci] + g[ci]
                s3 = v[2 * ci + 1]
            acc += w[i, 0] * s1 + w[i, 1] * s2 + w[i, 2] * s3
        out[base + b] = acc


def make_r1(x_core):
    """Row sums of the final batch of a core's slice, as [128, RB]."""
    rs = x_core[BPC - 1].astype(np.float32).sum(axis=1)  # (N,)
    return np.ascontiguousarray(rs.reshape(RB, 128).T)


def kernel(x, coef):
    x = np.ascontiguousarray(x, dtype=np.float32)
    coef = np.asarray(coef, dtype=np.float32)
    B = x.shape[0]
    assert B == BPC * NCORES and x.shape[1:] == (N, N)

    nc = get_nc()
    xt = np.ascontiguousarray(x.transpose(0, 2, 1))
    in_maps = [
        {
            "x": x[c * BPC : (c + 1) * BPC],
            "xt": xt[c * BPC : (c + 1) * BPC],
            "r1": make_r1(x[c * BPC : (c + 1) * BPC]),
        }
        for c in range(NCORES)
    ]
    res = run_bass_kernel_spmd(nc, in_maps, list(range(NCORES))).results

    out = np.zeros(B, dtype=np.float64)
    for c in range(NCORES):
        combine_partials(
            res[c]["acc_s"],
            res[c]["acc_v"],
            res[c]["acc_gs"],
            res[c]["acc_u"],
            coef,
            out,
            c * BPC,
        )
    return out.astype(np.float32)
